# revision 25
# baseline (speedup 1.0000x reference)
"""MoE experts FFN kernel for Trainium2 (8 NeuronCores, expert parallel).

Reference computation (per expert e of 8):
    inter = hidden_states[e] @ gate_up_w[e]        # (C,H)@(H,2I) -> (C,2I)
    gate, up = split(inter, 2, axis=-1)
    act = silu(gate) * up                          # (C,I)
    out[e] = act @ down_w[e]                       # (C,I)@(I,H) -> (C,H)

E == n_cores == 8, so each core owns one expert end-to-end (no collectives).

Device layout: everything transposed so the PE's contraction dim sits on
partitions for both operands (lhsT = weights stationary, rhs = Xt moving):

    interT[f, c] = sum_h Wgu[h, f] * Xt[h, c]
    outT[h, c]   = sum_i Wd[i, h] * actT[i, c]

The classical kernel streams 4224 N=512 matmuls at the PE's 1-column/cycle
floor (96.5% MFU) -- the only way faster is fewer matmuls.  mm1 (2/3 of the
MACs) uses one level of Strassen over (F2, H, C); the F2 split lands exactly
on the gate|up boundary:

    M1 = (A11+A22)(B11+B22)   M2 = (A21+A22)B11    M3 = A11(B12-B22)
    M4 = A22(B21-B11)         M5 = (A11+A12)B22    M6 = (A21-A11)(B11+B12)
    M7 = (A12-A22)(B21+B22)

    gateT[:, c<1024] = M1+M4-M5+M7      gateT[:, c>=1024] = M3+M5
    upT[:, c<1024]   = M2+M4            upT[:, c>=1024]   = M1-M2+M3+M6

7/8 of the matmuls: 22 f-blocks x 7 products x (16 k-tile MMs across two
PSUM banks).  Both operand combos are PRECOMPUTED ON HOST and DMA'd in
(they are pure functions of the inputs), so the device does only matmuls,
PSUM drains (fused +/- on the Vector engine), Silu-GLU, and DMA.  act
spills to DRAM scratch (SBUF holds the 7 Xt-combos instead) and reloads in
512-column chunks for a c-outer classical mm2.
"""

from contextlib import ExitStack

import numpy as np
import ml_dtypes

E, C, H, I = 8, 2048, 2048, 2816
F2 = 2 * I          # fused gate+up columns
P = 128             # partitions
NF = 512            # PSUM bank of fp32; mm chains write [P, 2*NF] tiles
KT2 = (H // 2) // P  # 8 k-tiles per H-half (Strassen mm1)
FB = I // P         # 22 f-blocks (gate tile i pairs with up tile i)
IT = I // P         # 22 i-tiles over I (mm2 contraction)
HT = H // P         # 16 h-tiles over H (mm2 output)
CT = C // NF        # 4 c-chunks of 512
CH = C // 2         # 1024, Strassen c-half
IT2 = (I // 2) // P  # 11 k-tiles per I-half (Strassen mm2)

_NC_CACHE = {}


def _build_nc(compute="float16"):
    if compute in _NC_CACHE:
        return _NC_CACHE[compute]

    import concourse.bacc as bacc
    import concourse.tile as tile
    from concourse import mybir

    cdt = getattr(mybir.dt, compute)
    f32 = mybir.dt.float32
    AFT = mybir.ActivationFunctionType
    ALU = mybir.AluOpType

    nc = bacc.Bacc(None, target_bir_lowering=False, name="moe_expert_ffn")

    # xt raw (combos formed on DVE: startup is DMA-bound, so ship the
    # smaller raw tensor); weight combos precomputed on host.
    xt_d = nc.dram_tensor("xt", [H, C], cdt, kind="ExternalInput")
    wc_d = nc.dram_tensor("wc", [FB, P, 7, KT2, P], cdt, kind="ExternalInput")
    wd_d = nc.dram_tensor("wd", [HT // 2, P, 7, IT2, P], cdt, kind="ExternalInput")
    odt = cdt if compute == "float16" else f32
    outT_d = nc.dram_tensor("outT", [H, C], odt, kind="ExternalOutput")

    xt_r = xt_d.ap().rearrange("(kt p) c -> p kt c", p=P)       # [128, 16, C]
    wc_a = wc_d.ap()
    wd_a = wd_d.ap()
    outT_a = outT_d.ap()

    with tile.TileContext(nc) as tc, ExitStack() as ctx:
        singles = ctx.enter_context(tc.tile_pool(name="singles", bufs=1))
        wpool = ctx.enter_context(tc.tile_pool(name="wpool", bufs=2))
        accp = ctx.enter_context(tc.tile_pool(name="accp", bufs=2))
        spool = ctx.enter_context(tc.tile_pool(name="spool", bufs=2))
        rpool = ctx.enter_context(tc.tile_pool(name="rpool", bufs=2))
        opool = ctx.enter_context(tc.tile_pool(name="opool", bufs=3))
        psum = ctx.enter_context(tc.tile_pool(name="psum", bufs=2, space="PSUM"))
        dram = ctx.enter_context(tc.tile_pool(name="dram", bufs=1, space="DRAM"))

        # DRAM scratch for the act spill (actT, one tile per i-block).
        act_d = dram.tile([FB, P, C], cdt, tag="actd", name="act_spill")

        # Dummy matmuls on zeroed tiles fill the PE's dead window while the
        # first inputs stream in (HAM un-throttles after ~3.4us of activity).
        wz = singles.tile([P, P], cdt, tag="wz", name="wz")
        xz = singles.tile([P, NF], cdt, tag="xz", name="xz")
        nc.vector.memset(wz, 0.0)
        nc.vector.memset(xz, 0.0)
        warm_ps = psum.tile([P, 2 * NF], f32, tag="pa", name="warm_ps")
        for w in range(24):
            nc.tensor.matmul(
                warm_ps[:, :NF], wz, xz, start=True, stop=True
            )

        # first two weight-combo blocks: highest DMA priority so f-blocks
        # 0/1 never wait on weights behind the xt stream
        wc0 = wpool.tile([P, 7, KT2, P], cdt, tag="wc", name="wc0")
        nc.sync.dma_start(out=wc0, in_=wc_a[0])
        wc1 = wpool.tile([P, 7, KT2, P], cdt, tag="wc", name="wc1")
        nc.sync.dma_start(out=wc1, in_=wc_a[1])

        # Xt combos formed per k-tile on the Vector engine as the raw
        # quadrant slices stream in; one tile per (product, kt) slice so
        # DMAs and readers don't false-share dependency state.
        bc = [[None] * KT2 for _ in range(7)]
        for m in range(7):
            for kt in range(KT2):
                bc[m][kt] = singles.tile(
                    [P, CH], cdt, tag=f"b{m}_{kt}", name=f"b{m}_{kt}"
                )
        for kt in range(KT2):
            nc.sync.dma_start(out=bc[1][kt], in_=xt_r[:, kt, 0:CH])        # B11
            nc.sync.dma_start(out=bc[4][kt], in_=xt_r[:, KT2 + kt, CH:C])  # B22
            t12 = spool.tile([P, CH], cdt, tag="q12", name=f"q12_{kt}")
            t21 = spool.tile([P, CH], cdt, tag="q21", name=f"q21_{kt}")
            nc.sync.dma_start(out=t12, in_=xt_r[:, kt, CH:C])              # B12
            nc.sync.dma_start(out=t21, in_=xt_r[:, KT2 + kt, 0:CH])        # B21
            nc.vector.tensor_add(bc[0][kt], bc[1][kt], bc[4][kt])  # B11+B22
            nc.vector.tensor_sub(bc[2][kt], t12, bc[4][kt])        # B12-B22
            nc.vector.tensor_sub(bc[3][kt], t21, bc[1][kt])        # B21-B11
            nc.vector.tensor_add(bc[5][kt], bc[1][kt], t12)        # B11+B12
            nc.vector.tensor_add(bc[6][kt], t21, bc[4][kt])        # B21+B22

        # per-product drain plan: (acc key, how); "sub" is acc - M fused
        plan = [
            [("gl", "copy"), ("uh", "copy")],          # M1
            [("ul", "copy"), ("uh", "sub")],           # M2
            [("gh", "copy"), ("uh", "add")],           # M3
            [("gl", "add"), ("ul", "add")],            # M4
            [("gh", "add"), ("gl", "sub")],            # M5
            [("uh", "add")],                           # M6
            [("gl", "add")],                           # M7
        ]

        # ---- mm1: per f-block, 7 Strassen products + drains + GLU ----
        for i in range(FB):
            if i == 0:
                wc_t = wc0
            elif i == 1:
                wc_t = wc1
            else:
                wc_t = wpool.tile([P, 7, KT2, P], cdt, tag="wc", name=f"wc{i}")
                nc.sync.dma_start(out=wc_t, in_=wc_a[i])

            acc = {}
            for nm in ("gl", "gh", "ul", "uh"):
                acc[nm] = accp.tile([P, 2 * NF], cdt, tag=nm, name=f"{nm}_{i}")

            def drain(m, ps):
                for key, how in plan[m]:
                    a = acc[key]
                    if how == "copy":
                        if m == 0:
                            # M1's copies go to the idle Scalar engine (it
                            # reads PSUM); no other engine touches M1's
                            # banks, and it keeps the copy off DVE's queue
                            # so the next block's PSUM recycling never
                            # waits behind DVE backlog.
                            nc.scalar.activation(out=a, in_=ps, func=AFT.Copy)
                        else:
                            nc.vector.tensor_copy(out=a, in_=ps)
                    elif how == "add":
                        nc.vector.tensor_add(a, ps, a)
                    else:  # a = (ps * -1) + a
                        nc.vector.scalar_tensor_tensor(
                            out=a, in0=ps, scalar=-1.0, in1=a,
                            op0=ALU.mult, op1=ALU.add,
                        )

            def mm(ps, m, kt, j):
                nc.tensor.matmul(
                    ps[:, j * NF : (j + 1) * NF],
                    wc_t[:, m, kt, :],
                    bc[m][kt][:, j * NF : (j + 1) * NF],
                    start=(kt == 0),
                    stop=(kt == KT2 - 1),
                )

            if i == 0:
                # kt-major waves: the PE starts the moment the first k-tile
                # combos exist instead of waiting for full products' worth
                # of Xt (startup is DMA-bound).
                pss = {}
                for wave in ((0, 1, 2, 3), (4, 5, 6)):
                    for m in wave:
                        pss[m] = psum.tile(
                            [P, 2 * NF], f32, tag="pa" if m % 2 == 0 else "pb",
                            name=f"m{m}_{i}",
                        )
                    for kt in range(KT2):
                        for m in wave:
                            for j in range(2):
                                mm(pss[m], m, kt, j)
                for m in range(7):
                    drain(m, pss[m])
            else:
                for m in range(7):
                    ps = psum.tile(
                        [P, 2 * NF], f32, tag="pa" if m % 2 == 0 else "pb",
                        name=f"m{m}_{i}",
                    )
                    for j in range(2):
                        for kt in range(KT2):
                            mm(ps, m, kt, j)
                    drain(m, ps)

            act_t = spool.tile([P, C], cdt, tag="act", name=f"act{i}")
            for half, g, u in ((0, "gl", "ul"), (1, "gh", "uh")):
                s_sb = spool.tile([P, 2 * NF], cdt, tag=f"sig{half}", name=f"sig{half}_{i}")
                nc.scalar.activation(out=s_sb, in_=acc[g], func=AFT.Silu)
                nc.vector.tensor_mul(
                    act_t[:, half * CH : (half + 1) * CH], s_sb, acc[u]
                )
            nc.sync.dma_start(out=act_d[i], in_=act_t)

        # ---- mm2: Strassen over (H, I, Cg), two c-groups of 1024 ----
        # act combos [I/2, 512] stream from the spill as (kt-pair) tiles
        # that reuse the Xt-combo tag buffers; the tag-free cascade off
        # mm1's last f-block (and off the previous group's last h-pair)
        # overlaps formation with compute.  Wd combos are host-packed.
        for g in range(2):
            g0 = g * CH
            # group 1's first two products use the 14 spare b-tags (free
            # since mm1's end), so their combos form during group 0's
            # compute instead of waiting for group 0's last h-pair.
            acomb = [[None] * 6 for _ in range(7)]
            for m in range(7):
                for t in range(6):
                    ti = (42 + m * 6 + t) if (g == 1 and m < 2) else m * 6 + t
                    acomb[m][t] = singles.tile(
                        [P, 2, NF], cdt, tag=f"b{ti // KT2}_{ti % KT2}",
                        name=f"ac{m}_{t}_{g}",
                    )

            def acf(m, kt):
                return acomb[m][kt // 2][:, kt % 2, :]

            # prefetch the first two Wd-combo blocks ahead of the act-combo
            # formation burst so their transfers don't contend at the
            # group boundary
            wds_pre = {}
            for j in range(2):
                wds_pre[j] = wpool.tile(
                    [P, 7, IT2, P], cdt, tag="wc", name=f"wds{j}_{g}"
                )
                nc.sync.dma_start(out=wds_pre[j], in_=wd_a[j])

            for kt in range(IT2):
                nc.sync.dma_start(
                    out=acf(1, kt), in_=act_d[kt, :, g0 : g0 + NF]
                )
                nc.sync.dma_start(
                    out=acf(4, kt), in_=act_d[IT2 + kt, :, g0 + NF : g0 + CH]
                )
                t12 = spool.tile([P, NF], cdt, tag="q12", name=f"a12_{kt}_{g}")
                t21 = spool.tile([P, NF], cdt, tag="q21", name=f"a21_{kt}_{g}")
                nc.sync.dma_start(out=t12, in_=act_d[kt, :, g0 + NF : g0 + CH])
                nc.sync.dma_start(out=t21, in_=act_d[IT2 + kt, :, g0 : g0 + NF])
                nc.vector.tensor_add(acf(0, kt), acf(1, kt), acf(4, kt))
                nc.vector.tensor_sub(acf(2, kt), t12, acf(4, kt))
                nc.vector.tensor_sub(acf(3, kt), t21, acf(1, kt))
                nc.vector.tensor_add(acf(5, kt), acf(1, kt), t12)
                nc.vector.tensor_add(acf(6, kt), t21, acf(4, kt))

            for j in range(HT // 2):
                if j < 2:
                    wds_t = wds_pre[j]
                else:
                    wds_t = wpool.tile(
                        [P, 7, IT2, P], cdt, tag="wc", name=f"wds{j}_{g}"
                    )
                    nc.sync.dma_start(out=wds_t, in_=wd_a[j])
                oacc = {
                    nm: accp.tile([P, NF], odt, tag=nm, name=f"o{nm}{j}_{g}")
                    for nm in ("gl", "gh", "ul", "uh")
                }
                for m in range(7):
                    ps = psum.tile(
                        [P, 2 * NF], f32, tag="pa" if m % 2 == 0 else "pb",
                        name=f"o{m}_{j}_{g}",
                    )
                    for kt in range(IT2):
                        nc.tensor.matmul(
                            ps[:, :NF],
                            wds_t[:, m, kt, :],
                            acf(m, kt),
                            start=(kt == 0),
                            stop=(kt == IT2 - 1),
                        )
                    for key, how in plan[m]:
                        a = oacc[key]
                        if how == "copy":
                            if m == 0:
                                nc.scalar.activation(
                                    out=a, in_=ps[:, :NF], func=AFT.Copy
                                )
                            else:
                                nc.vector.tensor_copy(out=a, in_=ps[:, :NF])
                        elif how == "add":
                            nc.vector.tensor_add(a, ps[:, :NF], a)
                        else:
                            nc.vector.scalar_tensor_tensor(
                                out=a, in0=ps[:, :NF], scalar=-1.0, in1=a,
                                op0=ALU.mult, op1=ALU.add,
                            )
                for nm, hrow, chalf in (
                    ("gl", j, 0), ("gh", j, 1),
                    ("ul", j + HT // 2, 0), ("uh", j + HT // 2, 1),
                ):
                    h0 = hrow * P
                    c0 = g0 + chalf * NF
                    nc.sync.dma_start(
                        out=outT_a[h0 : h0 + P, c0 : c0 + NF], in_=oacc[nm]
                    )

    nc.compile()
    _NC_CACHE[compute] = nc
    return nc


def _np_dtype(compute):
    return {"bfloat16": ml_dtypes.bfloat16, "float16": np.float16, "float32r": np.float32}[compute]


def _pack_wds(wd):
    """[I, H] -> Strassen stationary combos [HT//2, P, 7, IT2, P]."""
    out = np.empty((HT // 2, P, 7, IT2, P), dtype=wd.dtype)
    I2 = I // 2
    for j in range(HT // 2):
        hj = wd[:, j * P : (j + 1) * P]                      # h-low cols
        hj8 = wd[:, H // 2 + j * P : H // 2 + (j + 1) * P]   # h-high cols
        q0, q1 = hj[:I2], hj[I2:]
        q2, q3 = hj8[:I2], hj8[I2:]
        combos = (q0 + q3, q2 + q3, q0, q3, q0 + q1, q2 - q0, q1 - q3)
        for m, cb in enumerate(combos):
            # [1408, 128] -> [kt, p, f] -> [p, kt, f]
            out[j, :, m] = cb.reshape(IT2, P, P).transpose(1, 0, 2)
    return out


def _pack_wc(wgu):
    """[H, F2] -> Strassen stationary combos [FB, P, 7, KT2, P]."""
    out = np.empty((FB, P, 7, KT2, P), dtype=wgu.dtype)
    for i in range(FB):
        g = wgu[:, i * P : (i + 1) * P]            # [H, 128] gate cols
        u = wgu[:, I + i * P : I + (i + 1) * P]    # [H, 128] up cols
        g_lo, g_hi = g[: H // 2], g[H // 2 :]
        u_lo, u_hi = u[: H // 2], u[H // 2 :]
        combos = (g_lo + u_hi, u_lo + u_hi, g_lo, u_hi, g_lo + g_hi,
                  u_lo - g_lo, g_hi - u_hi)
        for m, cb in enumerate(combos):
            # [1024, 128] -> [kt, p, f] -> [p, kt, f]
            out[i, :, m] = cb.reshape(KT2, P, P).transpose(1, 0, 2)
    return out


def make_in_maps(hidden_states, gate_up_w, down_w, compute="float16"):
    dt = _np_dtype(compute)
    in_maps = []
    for e in range(E):
        in_maps.append(
            {
                "xt": np.ascontiguousarray(hidden_states[e].T).astype(dt),
                "wc": _pack_wc(gate_up_w[e].astype(dt)),
                "wd": _pack_wds(down_w[e].astype(dt)),
            }
        )
    return in_maps


def run_hw(in_maps, compute="float16", trace=False, **kwargs):
    from concourse import bass_utils

    if trace:
        # local-only devloop: skip the artifact-bucket upload
        bass_utils.upload_artifacts = lambda tmpdir: f"local:{tmpdir}"
    nc = _build_nc(compute)
    return bass_utils.run_bass_kernel_spmd(
        nc, in_maps, core_ids=list(range(E)), trace=trace, **kwargs
    )


def kernel(hidden_states, gate_up_w, down_w):
    compute = "float16"
    hidden_states = np.asarray(hidden_states)
    gate_up_w = np.asarray(gate_up_w)
    down_w = np.asarray(down_w)
    in_maps = make_in_maps(hidden_states, gate_up_w, down_w, compute)
    res = run_hw(in_maps, compute)
    out = np.empty((E, C, H), dtype=np.float32)
    for e in range(E):
        out[e] = res.results[e]["outT"].T
    return out
